# revision 1
# baseline (speedup 1.0000x reference)
"""Trainium2 Bass kernel for nn_AttentionBlock (b=4, c=512, h=w=64).

Sharding: 8 cores = (batch 0..3) x (sequence half 0..1). Each core receives
its batch's x [512, 4096] ROTATED so that the core's query half occupies
local columns 0:2048 (attention is permutation-invariant over keys, and
groupnorm stats are order-invariant, so one SPMD program serves all cores).

Per-core pipeline:
  A) groupnorm stats split across DVE (bn_stats) and ACT (Identity/Square
     with accum_out), + tiny indicator matmuls for the cross-partition group
     reduce/broadcast -> per-channel scale/bias
  B) stream x in 1024-col pairs: normalize on DVE, then 1x1-conv matmuls
     producing K [c, n] (per-chunk tiles) and V^T [n, c] in SBUF; Q (scaled
     by 1/sqrt(c)) spills to per-chunk DRAM scratch tiles
  C) attention per 512-query chunk: S^T tiles = K^T Q (PSUM), exp on ACT
     straight out of PSUM, U[c-tile] += V^T E on PE while Z accumulates E on
     DVE; Z broadcast via an all-ones matmul, 1/Z via reciprocal_approx_fast,
     folded in after the proj matmul together with bias + residual x.
All heavy matmuls run as float32r (full PE rate at 512-wide moving dim);
every tile feeding an fp32r matmul is produced with an f32r-typed output
(walrus requires the producer instruction to round to f32r).
"""

import os
import sys
from contextlib import ExitStack

for _p in ("/opt/trn_rl_repo", "/root/.axon_site/_ro/trn_rl_repo"):
    if os.path.isdir(_p) and _p not in sys.path:
        sys.path.insert(0, _p)

import numpy as np

import concourse.bass as bass
import concourse.tile as tile
from concourse.tile_rust import add_dep_helper
from concourse import bacc, mybir
from concourse.bass_utils import run_bass_kernel_spmd

F32 = mybir.dt.float32
F32R = mybir.dt.float32r
ALU = mybir.AluOpType
ACT = mybir.ActivationFunctionType

N_CORES = 8
C = 512          # channels
N = 4096         # h*w
NQ = 2048        # queries per core
CT = C // 128    # 4 channel tiles
NCHUNK = N // 512   # 8 column chunks
QCHUNK = NQ // 512  # 4 query chunks per core
MT = N // 128    # 32 key tiles
GSIZE = 16       # channels per group
EPS = 1e-5


def _r(ap):
    return ap.bitcast(F32R)


def build_module(reps: int = 1):
    nc = bacc.Bacc("TRN2", target_bir_lowering=False, debug=False,
                   num_devices=N_CORES)

    xin = nc.dram_tensor("xin", [C, N], F32, kind="ExternalInput").ap()
    wqkv = nc.dram_tensor("wqkv", [C, 3 * C], F32,
                          kind="ExternalInput").ap()
    wp = nc.dram_tensor("wp", [C, C], F32, kind="ExternalInput").ap()
    cvec = nc.dram_tensor("cvec", [128, 28 + C], F32,
                          kind="ExternalInput").ap()
    indb = nc.dram_tensor("indb", [8, 128], F32, kind="ExternalInput").ap()
    out = nc.dram_tensor("out", [C, NQ], F32, kind="ExternalOutput").ap()

    with tile.TileContext(nc) as tc, \
            nc.allow_low_precision(reason="float32r is 4 bytes"):
        for rep in range(reps):
            _emit_body(tc, rep, xin, wqkv, wp, cvec, indb, out)
    nc.compile()
    return nc


def _emit_body(tc, rep, xin, wqkv, wp, cvec, indb, out):
    nc = tc.nc
    with ExitStack() as ctx:
        # ---- persistent pools for this body ----
        const = ctx.enter_context(tc.tile_pool(name=f"const{rep}", bufs=1))
        kpool = ctx.enter_context(tc.tile_pool(name=f"kbuf{rep}", bufs=1))
        vpool = ctx.enter_context(tc.tile_pool(name=f"vbuf{rep}", bufs=1))
        statp = ctx.enter_context(tc.tile_pool(name=f"stat{rep}", bufs=1))
        qdp = ctx.enter_context(
            tc.tile_pool(name=f"qd{rep}", bufs=1, space="DRAM"))
        qscratch = [None] + [
            qdp.tile([C, 512], F32R, name=f"qsc{qc}", tag=f"qsc{qc}")
            for qc in range(1, QCHUNK)]
        q0p = ctx.enter_context(tc.tile_pool(name=f"q0p{rep}", bufs=1))
        q0_sb = [q0p.tile([128, 512], F32R, name=f"q0_{ct}", tag=f"q0_{ct}")
                 for ct in range(CT)]

        K_sb = [[kpool.tile([128, 512], F32R, name=f"K{ct}_{jc}", tag=f"K{ct}_{jc}")
                 for jc in range(NCHUNK)] for ct in range(CT)]
        V_sb = [vpool.tile([128, C], F32R, name=f"V{mt}", tag=f"V{mt}") for mt in range(MT)]

        cvec_t = const.tile([128, 28 + C], F32, name="cvec", tag="cvec")
        nc.sync.dma_start(cvec_t[:], cvec)
        indb_t = const.tile([8, 128], F32, name="indb", tag="indb")
        nc.sync.dma_start(indb_t[:], indb)
        ones_mat_f = const.tile([128, 128], F32, name="onemf", tag="onemf")
        nc.vector.memset(ones_mat_f[:], 1.0)
        ones_mat = const.tile([128, 128], F32R, name="onem", tag="onem")
        nc.vector.tensor_copy(ones_mat[:], ones_mat_f[:])
        eps_t = const.tile([128, 1], F32, name="epst", tag="epst")
        nc.vector.memset(eps_t[:], EPS)
        bq_t = [cvec_t[:, ct * 5 + 0:ct * 5 + 1] for ct in range(CT)]
        bk_t = [cvec_t[:, ct * 5 + 1:ct * 5 + 2] for ct in range(CT)]
        bp_t = [cvec_t[:, ct * 5 + 2:ct * 5 + 3] for ct in range(CT)]
        gnw_t = [cvec_t[:, ct * 5 + 3:ct * 5 + 4] for ct in range(CT)]
        gnb_t = [cvec_t[:, ct * 5 + 4:ct * 5 + 5] for ct in range(CT)]
        indr_t = cvec_t[:, 20:28]
        bvb_t = cvec_t[:, 28:28 + C]

        scale_c = [statp.tile([128, 1], F32, name=f"sc{ct}", tag=f"sc{ct}") for ct in range(CT)]
        bias_c = [statp.tile([128, 1], F32, name=f"bc{ct}", tag=f"bc{ct}") for ct in range(CT)]

        # ================= Phase A: groupnorm statistics =================
        with ExitStack() as pa:
            xa = pa.enter_context(tc.tile_pool(name=f"xa{rep}", bufs=1))
            tmpa = pa.enter_context(tc.tile_pool(name=f"tmpa{rep}", bufs=2))
            psa = pa.enter_context(
                tc.tile_pool(name=f"psa{rep}", bufs=2, space="PSUM"))

            # ct 0 takes the slower ACT accum_out path (its loads land
            # first); cts 1-3 take DVE bn_stats.
            stats = {ct: statp.tile([128, NCHUNK, 6], F32, name=f"st{ct}",
                                    tag=f"st{ct}") for ct in (1, 2, 3)}
            sacc = {0: statp.tile([128, 4], F32, name="sa0", tag="sa0")}
            last_a_load = None
            for ct in range(CT):
                for jj in range(2):  # [128, 2048] loads, row-split across
                    # the HWDGE (sync) and SWDGE (gpsimd) DMA paths
                    xt = xa.tile([128, 2048], F32, name="xat", tag="xat", bufs=8)
                    nc.gpsimd.dma_start(
                        xt[0:64, :], xin[ct * 128:ct * 128 + 64,
                                         jj * 2048:(jj + 1) * 2048])
                    last_a_load = nc.sync.dma_start(
                        xt[64:128, :], xin[ct * 128 + 64:(ct + 1) * 128,
                                           jj * 2048:(jj + 1) * 2048])
                    if ct != 0:
                        # DVE path: bn_stats
                        for kk in range(4):
                            nc.vector.bn_stats(
                                out=stats[ct][:, 4 * jj + kk, :],
                                in_=xt[:, kk * 512:(kk + 1) * 512])
                    else:
                        # ACT path: S1/S2 via activation accum_out, in place
                        # (Identity rewrites x unchanged; Square destroys x
                        # after its last use)
                        nc.scalar.activation(
                            out=xt[:], in_=xt[:], func=ACT.Identity,
                            accum_out=sacc[ct][:, 2 * jj:2 * jj + 1])
                        nc.scalar.activation(
                            out=xt[:], in_=xt[:], func=ACT.Square,
                            accum_out=sacc[ct][:, 2 * jj + 1:2 * jj + 2])
                t12 = tmpa.tile([128, 2], F32, name="t12", tag="t12")
                if ct != 0:
                    mv = tmpa.tile([128, 2], F32, name="mv", tag="mv")
                    nc.vector.bn_aggr(out=mv[:], in_=stats[ct][:])
                    # t12 = [mean, E[x^2]] per channel
                    nc.vector.tensor_copy(t12[:, 0:1], mv[:, 0:1])
                    nc.vector.tensor_mul(t12[:, 1:2], mv[:, 0:1], mv[:, 0:1])
                    nc.vector.tensor_add(t12[:, 1:2], t12[:, 1:2], mv[:, 1:2])
                else:
                    sa = sacc[ct]
                    nc.vector.tensor_add(t12[:, 0:1], sa[:, 0:1], sa[:, 2:3])
                    nc.vector.tensor_add(t12[:, 1:2], sa[:, 1:2], sa[:, 3:4])
                    nc.vector.tensor_scalar_mul(t12[:], t12[:], 1.0 / N)
                # group reduce: [8, 2] = indr^T @ t12   (indr holds 1/16)
                gps = psa.tile([8, 2], F32, name="gps", tag="gps")
                nc.tensor.matmul(gps[:], indr_t, t12[:],
                                 start=True, stop=True)
                g12 = tmpa.tile([8, 2], F32, name="g12", tag="g12")
                nc.vector.tensor_copy(g12[:], gps[:])
                # broadcast back to channels: [128, 2] = indb^T @ g12
                cps = psa.tile([128, 2], F32, name="cps", tag="cps")
                nc.tensor.matmul(cps[:], indb_t[:], g12[:],
                                 start=True, stop=True)
                # var = E[x^2] - mean^2 ; rstd = 1/sqrt(var+eps)
                cs = tmpa.tile([128, 2], F32, name="cs", tag="cs")
                nc.vector.tensor_copy(cs[:], cps[:])
                var_t = tmpa.tile([128, 1], F32, name="var", tag="var")
                nc.vector.tensor_mul(var_t[:], cs[:, 0:1], cs[:, 0:1])
                nc.vector.tensor_sub(var_t[:], cs[:, 1:2], var_t[:])
                sq_t = tmpa.tile([128, 1], F32, name="sq", tag="sq")
                nc.scalar.activation(out=sq_t[:], in_=var_t[:],
                                     func=ACT.Sqrt, bias=eps_t[:], scale=1.0)
                rstd_t = tmpa.tile([128, 1], F32, name="rstd", tag="rstd")
                nc.vector.reciprocal(rstd_t[:], sq_t[:])
                nc.vector.tensor_mul(scale_c[ct][:], rstd_t[:], gnw_t[ct])
                mt_t = tmpa.tile([128, 1], F32, name="mt", tag="mt")
                nc.vector.tensor_mul(mt_t[:], cs[:, 0:1], scale_c[ct][:])
                nc.vector.tensor_sub(bias_c[ct][:], gnb_t[ct], mt_t[:])

        # ================= Phase B: normalize + QKV =================
        with ExitStack() as pb:
            wpoolb = pb.enter_context(tc.tile_pool(name=f"wB{rep}", bufs=1))
            xb = pb.enter_context(tc.tile_pool(name=f"xb{rep}", bufs=2))
            qtp = pb.enter_context(tc.tile_pool(name=f"qt{rep}", bufs=3))
            psb = pb.enter_context(
                tc.tile_pool(name=f"psb{rep}", bufs=2, space="PSUM"))

            wq_t, wk_t, wv_t = [], [], []
            for ct in range(CT):
                wt = wpoolb.tile([128, 3 * C], F32R, name=f"w3{ct}",
                                 tag=f"w3{ct}")
                wd = nc.sync.dma_start(
                    wt[:], _r(wqkv[ct * 128:(ct + 1) * 128, :]))
                add_dep_helper(wd.ins, last_a_load.ins, sync=True,
                               reason="keep DMA rings clear for stats loads")
                wq_t.append(wt[:, 0:C])
                wk_t.append(wt[:, C:2 * C])
                wv_t.append(wt[:, 2 * C:3 * C])

            for jp in range(NCHUNK // 2):
                xw = []
                for ct in range(CT):
                    xt = xb.tile([128, 1024], F32R, name=f"xb{ct}", tag=f"xb{ct}")
                    xd = nc.sync.dma_start(
                        xt[:], _r(xin[ct * 128:(ct + 1) * 128,
                                      jp * 1024:(jp + 1) * 1024]))
                    if jp < 2:
                        add_dep_helper(xd.ins, last_a_load.ins, sync=True,
                                       reason="keep DMA rings clear for stats")
                    nc.vector.tensor_scalar(
                        out=xt[:], in0=xt[:],
                        scalar1=scale_c[ct][:],
                        scalar2=bias_c[ct][:], op0=ALU.mult, op1=ALU.add)
                    xw.append(xt)

                for jh in range(2):
                    j = jp * 2 + jh
                    xn = [xw[ct][:, jh * 512:(jh + 1) * 512] for ct in range(CT)]
                    # K chunk: K[o, j*512:...] += wk^T x
                    for ot in range(CT):
                        kps = psb.tile([128, 512], F32, name="kps", tag="kps")
                        for ct in range(CT):
                            nc.tensor.matmul(
                                kps[:],
                                _r(wk_t[ct][:, ot * 128:(ot + 1) * 128]),
                                _r(xn[ct][:]),
                                start=(ct == 0), stop=(ct == CT - 1))
                        nc.scalar.activation(
                            out=K_sb[ot][j][:], in_=kps[:],
                            func=ACT.Identity, bias=bk_t[ot], scale=1.0)
                    # V^T tiles: V^T[m-tile, :] += xn^T wv^T
                    for mti in range(4):
                        mt = j * 4 + mti
                        vps = psb.tile([128, C], F32, name="vps", tag="vps")
                        for ct in range(CT):
                            nc.tensor.matmul(
                                vps[:],
                                _r(xn[ct][:, mti * 128:(mti + 1) * 128]),
                                _r(wv_t[ct]),
                                start=(ct == 0), stop=(ct == CT - 1))
                        nc.vector.tensor_add(V_sb[mt][:], vps[:], bvb_t)
                    # Q chunk (only local columns 0:2048 are queries)
                    if j < QCHUNK:
                        for ot in range(CT):
                            qps = psb.tile([128, 512], F32, name="qps", tag="qps")
                            for ct in range(CT):
                                nc.tensor.matmul(
                                    qps[:],
                                    _r(wq_t[ct][:, ot * 128:(ot + 1) * 128]),
                                    _r(xn[ct][:]),
                                    start=(ct == 0), stop=(ct == CT - 1))
                            if j == 0:
                                nc.scalar.activation(
                                    out=q0_sb[ot][:], in_=qps[:],
                                    func=ACT.Identity,
                                    bias=bq_t[ot], scale=1.0)
                            else:
                                qt = qtp.tile([128, 512], F32R, name="qt",
                                              tag="qt")
                                nc.scalar.activation(out=qt[:], in_=qps[:],
                                                     func=ACT.Identity,
                                                     bias=bq_t[ot], scale=1.0)
                                nc.sync.dma_start(
                                    qscratch[j][ot * 128:(ot + 1) * 128, :],
                                    qt[:])

        # ================= Phase C: attention + proj =================
        with ExitStack() as pc:
            wpoolc = pc.enter_context(tc.tile_pool(name=f"wC{rep}", bufs=1))
            qcp = pc.enter_context(tc.tile_pool(name=f"qc{rep}", bufs=2))
            epool = pc.enter_context(tc.tile_pool(name=f"e{rep}", bufs=3))
            apool = pc.enter_context(tc.tile_pool(name=f"at{rep}", bufs=1))
            xrp = pc.enter_context(tc.tile_pool(name=f"xr{rep}", bufs=2))
            outp = pc.enter_context(tc.tile_pool(name=f"out{rep}", bufs=4))
            miscp = pc.enter_context(tc.tile_pool(name=f"mi{rep}", bufs=1))
            ps_s = pc.enter_context(
                tc.tile_pool(name=f"pss{rep}", bufs=3, space="PSUM"))
            ps_u = pc.enter_context(
                tc.tile_pool(name=f"psu{rep}", bufs=1, space="PSUM"))

            wp_t = []
            for ct in range(CT):
                t = wpoolc.tile([128, C], F32R, name=f"wp{ct}", tag=f"wp{ct}")
                nc.sync.dma_start(t[:], _r(wp[ct * 128:(ct + 1) * 128, :]))
                wp_t.append(t)

            for qc in range(QCHUNK):
                if qc == 0:
                    q_sb = q0_sb
                else:
                    q_sb = []
                    for ct in range(CT):
                        t = qcp.tile([128, 512], F32R, name=f"q{ct}",
                                     tag=f"q{ct}")
                        nc.sync.dma_start(
                            t[:], qscratch[qc][ct * 128:(ct + 1) * 128, :])
                        q_sb.append(t)
                xres = []
                for ot in range(CT):
                    t = xrp.tile([128, 512], F32, name=f"xr{ot}", tag=f"xr{ot}")
                    nc.sync.dma_start(
                        t[:], xin[ot * 128:(ot + 1) * 128,
                                  qc * 512:(qc + 1) * 512])
                    xres.append(t)

                u = [ps_u.tile([128, 512], F32, name=f"u{ct}", tag=f"u{ct}")
                     for ct in range(CT)]
                zacc = miscp.tile([128, 512], F32R, name="za", tag="za")

                # software pipeline: scores(mt+1) before PV(mt)
                s_tiles = {}
                e_tiles = {}

                def scores(mt):
                    s = ps_s.tile([128, 512], F32, name="s", tag="s")
                    for ct in range(CT):
                        nc.tensor.matmul(
                            s[:],
                            K_sb[ct][mt // 4][:, (mt % 4) * 128:
                                              (mt % 4 + 1) * 128],
                            q_sb[ct][:],
                            start=(ct == 0), stop=(ct == CT - 1))
                    e = epool.tile([128, 512], F32R, name="e", tag="e")
                    nc.scalar.activation(out=e[:], in_=s[:], func=ACT.Exp)
                    e_tiles[mt] = e

                def pv(mt):
                    e = e_tiles.pop(mt)
                    for ct in range(CT):
                        nc.tensor.matmul(
                            u[ct][:],
                            _r(V_sb[mt][:, ct * 128:(ct + 1) * 128]),
                            _r(e[:]),
                            start=(mt == 0), stop=(mt == MT - 1))
                    if mt == 0:
                        nc.vector.tensor_copy(zacc[:], e[:])
                    else:
                        nc.vector.tensor_add(zacc[:], zacc[:], e[:])

                scores(0)
                for mt in range(MT):
                    if mt + 1 < MT:
                        scores(mt + 1)
                    pv(mt)

                attn = []
                for ct in range(CT):
                    t = apool.tile([128, 512], F32R, name=f"a{ct}", tag=f"a{ct}")
                    nc.vector.tensor_copy(t[:], u[ct][:])
                    attn.append(t)

                # Z broadcast in one matmul: zb[m, n] = colsum(zacc)[n]
                zb = ps_s.tile([128, 512], F32, name="s", tag="s")
                nc.tensor.matmul(zb[:], ones_mat[:], zacc[:],
                                 start=True, stop=True)
                rb_sb = miscp.tile([128, 512], F32, name="rb", tag="rb")
                nc.vector.reciprocal_approx_fast(rb_sb[:], zb[:])

                for ot in range(CT):
                    pp = ps_s.tile([128, 512], F32, name="s", tag="s")
                    for ct in range(CT):
                        nc.tensor.matmul(
                            pp[:],
                            _r(wp_t[ct][:, ot * 128:(ot + 1) * 128]),
                            _r(attn[ct][:]),
                            start=(ct == 0), stop=(ct == CT - 1))
                    t_o = outp.tile([128, 512], F32, name="out", tag="out")
                    nc.vector.tensor_mul(t_o[:], pp[:], rb_sb[:])
                    nc.vector.scalar_tensor_tensor(
                        out=t_o[:], in0=t_o[:], scalar=bp_t[ot],
                        in1=xres[ot][:], op0=ALU.add, op1=ALU.add)
                    nc.sync.dma_start(
                        out[ot * 128:(ot + 1) * 128,
                            qc * 512:(qc + 1) * 512], t_o[:])


# ---------------- host-side sharding / gather ----------------

_CACHED_NC = None


def _get_nc():
    global _CACHED_NC
    if _CACHED_NC is None:
        _CACHED_NC = build_module(reps=1)
    return _CACHED_NC


def _make_in_maps(x, gn_w, gn_b, qkv_w, qkv_b, proj_w, proj_b):
    b, c, h, w = x.shape
    n = h * w
    assert (b, c, n) == (4, C, N)
    xr = np.ascontiguousarray(x.reshape(b, c, n)).astype(np.float32)
    scale = np.float32(1.0 / np.sqrt(np.float32(c)))

    wqkv_h = np.ascontiguousarray(np.concatenate(
        [qkv_w[0:c].T * scale, qkv_w[c:2 * c].T, qkv_w[2 * c:3 * c].T],
        axis=1)).astype(np.float32)
    wp_h = np.ascontiguousarray(proj_w.T).astype(np.float32)
    bq_h = (qkv_b[0:c] * scale).reshape(CT, 128)
    bk_h = qkv_b[c:2 * c].reshape(CT, 128)
    bp_h = proj_b.reshape(CT, 128)
    gnw_h = gn_w.reshape(CT, 128)
    gnb_h = gn_b.reshape(CT, 128)
    pidx = np.arange(128)
    indr_h = (pidx[:, None] // GSIZE == np.arange(8)[None, :]).astype(
        np.float32) / GSIZE
    indb_h = (np.arange(8)[:, None] == pidx[None, :] // GSIZE).astype(
        np.float32)
    cvec_h = np.zeros((128, 28 + c), np.float32)
    for ct in range(CT):
        for q, vec in enumerate((bq_h, bk_h, bp_h, gnw_h, gnb_h)):
            cvec_h[:, ct * 5 + q] = vec[ct]
    cvec_h[:, 20:28] = indr_h
    cvec_h[:, 28:28 + c] = np.broadcast_to(qkv_b[2 * c:3 * c], (128, c))

    shared = dict(wqkv=wqkv_h, wp=wp_h, cvec=cvec_h,
                  indb=indb_h)
    in_maps = []
    for core in range(N_CORES):
        bi, half = core // 2, core % 2
        xb = xr[bi]
        if half:
            xb = np.ascontiguousarray(
                np.concatenate([xb[:, NQ:], xb[:, :NQ]], axis=1))
        in_maps.append({"xin": xb, **shared})
    return in_maps


def kernel(x, gn_w, gn_b, qkv_w, qkv_b, proj_w, proj_b):
    nc = _get_nc()
    in_maps = _make_in_maps(x, gn_w, gn_b, qkv_w, qkv_b, proj_w, proj_b)
    res = run_bass_kernel_spmd(nc, in_maps, list(range(N_CORES)))
    b, c, h, w = x.shape
    out_full = np.empty((b, C, N), dtype=np.float32)
    for core in range(N_CORES):
        bi, half = core // 2, core % 2
        out_full[bi, :, half * NQ:(half + 1) * NQ] = res.results[core]["out"]
    return out_full.reshape(b, c, h, w)



# revision 5
# speedup vs baseline: 1.6862x; 1.6862x over previous
"""Trainium2 Bass kernel for nn_AttentionBlock (b=4, c=512, h=w=64).

Sharding: 8 cores = (batch 0..3) x (sequence half 0..1). Each core receives
its batch's x [512, 4096] ROTATED so that the core's query half occupies
local columns 0:2048 (attention is permutation-invariant over keys, and
groupnorm stats are order-invariant, so one SPMD program serves all cores).

Per-core pipeline (fp8e4 + DoubleRow everywhere on the PE):
  A) x loaded ONCE into resident SBUF tiles [128, 2048] x 8; groupnorm
     stats split across DVE (bn_stats, cts 1-3) and ACT (Identity/Square
     accum_out, ct 0), tiny indicator matmuls for the cross-partition group
     reduce/broadcast -> per-channel scale/bias.
  B) normalize x from SBUF into fp8 pair tiles [128, 2, 1024] (DVE), then
     1x1-conv QKV as fp8 DoubleRow matmuls (contraction 512 = 2 pairs of
     2x128): K -> fp8 pair tiles [128,2,512] per column chunk, V^T -> fp8
     pair tiles (key-tile pairs), Q -> fp8 pair tiles, all resident.
     Weights are prescaled x16 on host (fp8e4 floor); the ACT that reads
     PSUM applies 1/16 (+ conv bias) while rounding to fp8.
  C) attention per 512-query chunk: S^T pair = K^T Q (2 DR matmuls/key
     tile), exp on ACT with scale=1/sqrt(c), bias=-1.5 (shift cancels in
     softmax; keeps E well under fp8e4 max 240) -> fp8 E pair tiles;
     U[ct] += V^T E as DR matmuls over 16 key-tile pairs; Z accumulated on
     DVE over E pairs, column-summed+broadcast via a 1/8-valued matmul
     (folds an x8 attn prescale); attn = U * (8/Z) in fp8; proj as DR
     matmuls; post-proj ACT applies 1/128 + bias (incl. host-folded
     proj_w @ v_bias term), DVE adds the residual straight from the
     resident x tiles.
HBM traffic: x read once (8MB), weights ~1MB, out 4MB. No DRAM scratch.
"""

import os
import sys
from contextlib import ExitStack

for _p in ("/opt/trn_rl_repo", "/root/.axon_site/_ro/trn_rl_repo"):
    if os.path.isdir(_p) and _p not in sys.path:
        sys.path.insert(0, _p)

import numpy as np
import ml_dtypes

import concourse.bass as bass
import concourse.tile as tile
from concourse.tile_rust import add_dep_helper
from concourse import bacc, mybir
from concourse.bass_utils import run_bass_kernel_spmd

F32 = mybir.dt.float32
F32R = mybir.dt.float32r
F8 = mybir.dt.float8e4
NP8 = ml_dtypes.float8_e4m3
ALU = mybir.AluOpType
ACT = mybir.ActivationFunctionType
DR = mybir.MatmulPerfMode.DoubleRow

N_CORES = 8
C = 512          # channels
N = 4096         # h*w
NQ = 2048        # queries per core
CT = C // 128    # 4 channel tiles
NCHUNK = N // 512   # 8 column chunks
QCHUNK = NQ // 512  # 4 query chunks per core
MT = N // 128    # 32 key tiles
PRS = MT // 2    # 16 key-tile pairs
GSIZE = 16       # channels per group
EPS = 1e-5
WSCALE = 16.0    # host-side fp8 weight prescale
ASCALE = 8.0     # attn prescale (folded into the Z broadcast matmul)
ESHIFT = -1.5    # exp shift; cancels in softmax
SCALE_QK = 1.0 / float(np.sqrt(np.float32(C)))


def build_module(reps: int = 1):
    nc = bacc.Bacc("TRN2", target_bir_lowering=False, debug=False,
                   num_devices=N_CORES)

    xin = nc.dram_tensor("xin", [C, N], F32, kind="ExternalInput").ap()
    w8 = nc.dram_tensor("w8", [C, 3 * C], F8, kind="ExternalInput").ap()
    wpd = nc.dram_tensor("wpd", [C, C], F8, kind="ExternalInput").ap()
    cvec = nc.dram_tensor("cvec", [128, 28], F32, kind="ExternalInput").ap()
    indb = nc.dram_tensor("indb", [8, 128], F32, kind="ExternalInput").ap()
    out = nc.dram_tensor("out", [C, NQ], F32, kind="ExternalOutput").ap()

    with tile.TileContext(nc) as tc, \
            nc.allow_low_precision(reason="fp8 attention by design"):
        for rep in range(reps):
            _emit_body(tc, rep, xin, w8, wpd, cvec, indb, out)
    nc.compile()
    return nc


def _emit_body(tc, rep, xin, w8, wpd, cvec, indb, out):
    nc = tc.nc
    with ExitStack() as ctx:
        # ---- persistent pools ----
        const = ctx.enter_context(tc.tile_pool(name=f"const{rep}", bufs=1))
        xpool = ctx.enter_context(tc.tile_pool(name=f"xres{rep}", bufs=1))
        kpool = ctx.enter_context(tc.tile_pool(name=f"kbuf{rep}", bufs=1))
        vpool = ctx.enter_context(tc.tile_pool(name=f"vbuf{rep}", bufs=1))
        qpool = ctx.enter_context(tc.tile_pool(name=f"qbuf{rep}", bufs=1))
        wpool = ctx.enter_context(tc.tile_pool(name=f"wgt{rep}", bufs=1))
        statp = ctx.enter_context(tc.tile_pool(name=f"stat{rep}", bufs=1))

        # resident x: xa[ct*2 + jj] = [128, 2048] f32 (cols jj*2048...)
        xa = [xpool.tile([128, 2048], F32, name=f"xa{i}", tag=f"xa{i}")
              for i in range(8)]
        # fp8 K: pair p (channels 256p..256p+255), column chunk j
        K_f8 = [[kpool.tile([128, 2, 512], F8, name=f"K{p}_{j}",
                            tag=f"K{p}_{j}") for j in range(NCHUNK)]
                for p in range(2)]
        # fp8 V^T: key-tile pair pr, free = 512 channels
        V_f8 = [vpool.tile([128, 2, 512], F8, name=f"V{pr}", tag=f"V{pr}")
                for pr in range(PRS)]
        # fp8 Q: query chunk qc, channel pair p
        Q_f8 = [[qpool.tile([128, 2, 512], F8, name=f"Q{qc}_{p}",
                            tag=f"Q{qc}_{p}") for p in range(2)]
                for qc in range(QCHUNK)]
        # fp8 weights: qkv pair tiles and proj pair tiles
        w3 = [wpool.tile([128, 2, 3 * C], F8, name=f"w3_{p}", tag=f"w3_{p}")
              for p in range(2)]
        wp8 = [wpool.tile([128, 2, C], F8, name=f"wp{p}", tag=f"wp{p}")
               for p in range(2)]

        cvec_t = const.tile([128, 28], F32, name="cvec", tag="cvec")
        nc.sync.dma_start(cvec_t[:], cvec)
        indb_t = const.tile([8, 128], F32, name="indb", tag="indb")
        nc.sync.dma_start(indb_t[:], indb)
        ones_mat_f = const.tile([128, 128], F32, name="onemf", tag="onemf")
        nc.vector.memset(ones_mat_f[:], 1.0 / ASCALE)
        ones_mat = const.tile([128, 128], F32R, name="onem", tag="onem")
        nc.vector.tensor_copy(ones_mat[:], ones_mat_f[:])
        eps_t = const.tile([128, 1], F32, name="epst", tag="epst")
        nc.vector.memset(eps_t[:], EPS)
        esh_t = const.tile([128, 1], F32, name="esht", tag="esht")
        nc.vector.memset(esh_t[:], ESHIFT)
        bq_t = [cvec_t[:, ct * 5 + 0:ct * 5 + 1] for ct in range(CT)]
        bk_t = [cvec_t[:, ct * 5 + 1:ct * 5 + 2] for ct in range(CT)]
        bp_t = [cvec_t[:, ct * 5 + 2:ct * 5 + 3] for ct in range(CT)]
        gnw_t = [cvec_t[:, ct * 5 + 3:ct * 5 + 4] for ct in range(CT)]
        gnb_t = [cvec_t[:, ct * 5 + 4:ct * 5 + 5] for ct in range(CT)]
        indr_t = cvec_t[:, 20:28]

        scale_c = [statp.tile([128, 1], F32, name=f"sc{ct}", tag=f"sc{ct}")
                   for ct in range(CT)]
        bias_c = [statp.tile([128, 1], F32, name=f"bc{ct}", tag=f"bc{ct}")
                  for ct in range(CT)]

        # ================= Phase A: load x + groupnorm statistics ========
        with ExitStack() as pa:
            scpa = pa.enter_context(tc.tile_pool(name=f"sca{rep}", bufs=2))
            tmpa = pa.enter_context(tc.tile_pool(name=f"tmpa{rep}", bufs=2))
            psa = pa.enter_context(
                tc.tile_pool(name=f"psa{rep}", bufs=2, space="PSUM"))

            # ct 0 takes the ACT accum_out path; cts 1-3 take DVE bn_stats.
            stats = {ct: statp.tile([128, NCHUNK, 6], F32, name=f"st{ct}",
                                    tag=f"st{ct}") for ct in (1, 2, 3)}
            sacc = {0: statp.tile([128, 4], F32, name="sa0", tag="sa0")}
            last_a_load = None
            for ct in range(CT):
                for jj in range(2):  # [128, 2048] loads, row-split across
                    # the HWDGE (sync) and SWDGE (gpsimd) DMA paths
                    xt = xa[ct * 2 + jj]
                    nc.gpsimd.dma_start(
                        xt[0:64, :], xin[ct * 128:ct * 128 + 64,
                                         jj * 2048:(jj + 1) * 2048])
                    last_a_load = nc.sync.dma_start(
                        xt[64:128, :], xin[ct * 128 + 64:(ct + 1) * 128,
                                           jj * 2048:(jj + 1) * 2048])
                    if ct != 0:
                        for kk in range(4):
                            nc.vector.bn_stats(
                                out=stats[ct][:, 4 * jj + kk, :],
                                in_=xt[:, kk * 512:(kk + 1) * 512])
                    else:
                        # ACT path: S1/S2 via activation accum_out, writing
                        # the (discarded) value output to a scratch tile so
                        # the resident x stays pristine.
                        scr = scpa.tile([128, 2048], F32, name="scr",
                                        tag="scr")
                        nc.scalar.activation(
                            out=scr[:], in_=xt[:], func=ACT.Identity,
                            accum_out=sacc[ct][:, 2 * jj:2 * jj + 1])
                        nc.scalar.activation(
                            out=scr[:], in_=xt[:], func=ACT.Square,
                            accum_out=sacc[ct][:, 2 * jj + 1:2 * jj + 2])
                t12 = tmpa.tile([128, 2], F32, name="t12", tag="t12")
                if ct != 0:
                    mv = tmpa.tile([128, 2], F32, name="mv", tag="mv")
                    nc.vector.bn_aggr(out=mv[:], in_=stats[ct][:])
                    # t12 = [mean, E[x^2]] per channel
                    nc.vector.tensor_copy(t12[:, 0:1], mv[:, 0:1])
                    nc.vector.tensor_mul(t12[:, 1:2], mv[:, 0:1], mv[:, 0:1])
                    nc.vector.tensor_add(t12[:, 1:2], t12[:, 1:2], mv[:, 1:2])
                else:
                    sa = sacc[ct]
                    nc.vector.tensor_add(t12[:, 0:1], sa[:, 0:1], sa[:, 2:3])
                    nc.vector.tensor_add(t12[:, 1:2], sa[:, 1:2], sa[:, 3:4])
                    nc.vector.tensor_scalar_mul(t12[:], t12[:], 1.0 / N)
                # group reduce: [8, 2] = indr^T @ t12   (indr holds 1/16)
                gps = psa.tile([8, 2], F32, name="gps", tag="gps")
                nc.tensor.matmul(gps[:], indr_t, t12[:],
                                 start=True, stop=True)
                g12 = tmpa.tile([8, 2], F32, name="g12", tag="g12")
                nc.vector.tensor_copy(g12[:], gps[:])
                # broadcast back to channels: [128, 2] = indb^T @ g12
                cps = psa.tile([128, 2], F32, name="cps", tag="cps")
                nc.tensor.matmul(cps[:], indb_t[:], g12[:],
                                 start=True, stop=True)
                # var = E[x^2] - mean^2 ; rstd = 1/sqrt(var+eps)
                cs = tmpa.tile([128, 2], F32, name="cs", tag="cs")
                nc.vector.tensor_copy(cs[:], cps[:])
                var_t = tmpa.tile([128, 1], F32, name="var", tag="var")
                nc.vector.tensor_mul(var_t[:], cs[:, 0:1], cs[:, 0:1])
                nc.vector.tensor_sub(var_t[:], cs[:, 1:2], var_t[:])
                sq_t = tmpa.tile([128, 1], F32, name="sq", tag="sq")
                nc.scalar.activation(out=sq_t[:], in_=var_t[:],
                                     func=ACT.Sqrt, bias=eps_t[:], scale=1.0)
                rstd_t = tmpa.tile([128, 1], F32, name="rstd", tag="rstd")
                nc.vector.reciprocal(rstd_t[:], sq_t[:])
                nc.vector.tensor_mul(scale_c[ct][:], rstd_t[:], gnw_t[ct])
                mt_t = tmpa.tile([128, 1], F32, name="mt", tag="mt")
                nc.vector.tensor_mul(mt_t[:], cs[:, 0:1], scale_c[ct][:])
                nc.vector.tensor_sub(bias_c[ct][:], gnb_t[ct], mt_t[:])

        # weight loads (after the x DMAs on the sync ring)
        for p in range(2):
            for s in range(2):
                r0 = p * 256 + s * 128
                wd = nc.sync.dma_start(w3[p][:, s, :], w8[r0:r0 + 128, :])
                add_dep_helper(wd.ins, last_a_load.ins, sync=True,
                               reason="x loads first on the sync ring")
                wd = nc.sync.dma_start(wp8[p][:, s, :], wpd[r0:r0 + 128, :])
                add_dep_helper(wd.ins, last_a_load.ins, sync=True,
                               reason="x loads first on the sync ring")

        # ================= Phase B: normalize + QKV (fp8 DoubleRow) ======
        with ExitStack() as pb:
            xbp = pb.enter_context(tc.tile_pool(name=f"xb{rep}", bufs=2))
            psb = pb.enter_context(
                tc.tile_pool(name=f"psb{rep}", bufs=2, space="PSUM"))

            for jp in range(NCHUNK // 2):
                xf8 = []
                for p in range(2):
                    xt = xbp.tile([128, 2, 1024], F8, name=f"xf{p}",
                                  tag=f"xf{p}")
                    for s in range(2):
                        ct = 2 * p + s
                        src = xa[ct * 2 + jp // 2][
                            :, (jp % 2) * 1024:(jp % 2) * 1024 + 1024]
                        nc.vector.tensor_scalar(
                            out=xt[:, s, :], in0=src,
                            scalar1=scale_c[ct][:],
                            scalar2=bias_c[ct][:], op0=ALU.mult, op1=ALU.add)
                    xf8.append(xt)

                for jh in range(2):
                    j = jp * 2 + jh
                    xn = [xf8[p][:, :, jh * 512:(jh + 1) * 512]
                          for p in range(2)]
                    # K chunk
                    for ot in range(CT):
                        kps = psb.tile([128, 512], F32, name="kps", tag="kps")
                        for p in range(2):
                            nc.tensor.matmul(
                                kps[:],
                                w3[p][:, :, C + ot * 128:C + (ot + 1) * 128],
                                xn[p], start=(p == 0), stop=(p == 1),
                                perf_mode=DR)
                        nc.scalar.activation(
                            out=K_f8[ot // 2][j][:, ot % 2, :], in_=kps[:],
                            func=ACT.Identity, bias=bk_t[ot],
                            scale=1.0 / WSCALE)
                    # V^T tiles (x stationary, wv moving)
                    for mti in range(4):
                        mt = j * 4 + mti
                        vps = psb.tile([128, C], F32, name="vps", tag="vps")
                        for p in range(2):
                            nc.tensor.matmul(
                                vps[:],
                                xn[p][:, :, mti * 128:(mti + 1) * 128],
                                w3[p][:, :, 2 * C:3 * C],
                                start=(p == 0), stop=(p == 1), perf_mode=DR)
                        nc.vector.tensor_scalar_mul(
                            V_f8[mt // 2][:, mt % 2, :], vps[:], 1.0 / WSCALE)
                    # Q chunk (only local columns 0:2048 are queries)
                    if j < QCHUNK:
                        for ot in range(CT):
                            qps = psb.tile([128, 512], F32, name="qps",
                                           tag="qps")
                            for p in range(2):
                                nc.tensor.matmul(
                                    qps[:],
                                    w3[p][:, :, ot * 128:(ot + 1) * 128],
                                    xn[p], start=(p == 0), stop=(p == 1),
                                    perf_mode=DR)
                            nc.scalar.activation(
                                out=Q_f8[j][ot // 2][:, ot % 2, :],
                                in_=qps[:], func=ACT.Identity,
                                bias=bq_t[ot], scale=1.0 / WSCALE)

        # ================= Phase C: attention + proj (fp8 DoubleRow) =====
        with ExitStack() as pc:
            epool = pc.enter_context(tc.tile_pool(name=f"e{rep}", bufs=3))
            apool = pc.enter_context(tc.tile_pool(name=f"at{rep}", bufs=2))
            outp = pc.enter_context(tc.tile_pool(name=f"out{rep}", bufs=4))
            miscp = pc.enter_context(tc.tile_pool(name=f"mi{rep}", bufs=2))
            ps_s = pc.enter_context(
                tc.tile_pool(name=f"pss{rep}", bufs=4, space="PSUM"))
            ps_u = pc.enter_context(
                tc.tile_pool(name=f"psu{rep}", bufs=1, space="PSUM"))

            for qc in range(QCHUNK):
                u = [ps_u.tile([128, 512], F32, name=f"u{ct}", tag=f"u{ct}")
                     for ct in range(CT)]
                zacc = miscp.tile([128, 2, 512], F32, name="za", tag="za")

                e_tiles = {}

                def scores_pair(pr, qc=qc, e_tiles=e_tiles):
                    e = epool.tile([128, 2, 512], F8, name="e", tag="e")
                    for i2 in range(2):
                        mt = 2 * pr + i2
                        s = ps_s.tile([128, 512], F32, name="s", tag="s")
                        for p in range(2):
                            nc.tensor.matmul(
                                s[:],
                                K_f8[p][mt // 4][
                                    :, :, (mt % 4) * 128:(mt % 4 + 1) * 128],
                                Q_f8[qc][p][:], start=(p == 0), stop=(p == 1),
                                perf_mode=DR)
                        nc.scalar.activation(
                            out=e[:, i2, :], in_=s[:], func=ACT.Exp,
                            bias=esh_t[:], scale=SCALE_QK)
                    e_tiles[pr] = e

                def pv(pr, u=u, zacc=zacc, e_tiles=e_tiles):
                    e = e_tiles.pop(pr)
                    for ct in range(CT):
                        nc.tensor.matmul(
                            u[ct][:],
                            V_f8[pr][:, :, ct * 128:(ct + 1) * 128],
                            e[:], start=(pr == 0), stop=(pr == PRS - 1),
                            perf_mode=DR)
                    if pr == 0:
                        nc.vector.tensor_copy(zacc[:], e[:])
                    else:
                        nc.vector.tensor_add(zacc[:], zacc[:], e[:])

                scores_pair(0)
                for pr in range(PRS):
                    if pr + 1 < PRS:
                        scores_pair(pr + 1)
                    pv(pr)

                # Z: fold pair halves, then column-sum+broadcast via a
                # (1/ASCALE)-valued matmul; rbb = ASCALE / Z
                zh = miscp.tile([128, 512], F32R, name="zh", tag="zh")
                nc.vector.tensor_add(zh[:], zacc[:, 0, :], zacc[:, 1, :])
                zbp = ps_s.tile([128, 512], F32, name="s", tag="s")
                nc.tensor.matmul(zbp[:], ones_mat[:], zh[:],
                                 start=True, stop=True)
                rbb = miscp.tile([128, 512], F32, name="rb", tag="rb")
                nc.vector.reciprocal_approx_fast(rbb[:], zbp[:])

                attn8 = [apool.tile([128, 2, 512], F8, name=f"a{p}",
                                    tag=f"a{p}") for p in range(2)]
                for ct in range(CT):
                    nc.vector.tensor_mul(
                        attn8[ct // 2][:, ct % 2, :], u[ct][:], rbb[:])

                for ot in range(CT):
                    pp = ps_s.tile([128, 512], F32, name="s", tag="s")
                    for p in range(2):
                        nc.tensor.matmul(
                            pp[:], wp8[p][:, :, ot * 128:(ot + 1) * 128],
                            attn8[p][:], start=(p == 0), stop=(p == 1),
                            perf_mode=DR)
                    t_o = outp.tile([128, 512], F32, name="out", tag="out")
                    nc.scalar.activation(
                        out=t_o[:], in_=pp[:], func=ACT.Identity,
                        bias=bp_t[ot], scale=1.0 / (WSCALE * ASCALE))
                    nc.vector.tensor_add(
                        t_o[:], t_o[:],
                        xa[ot * 2][:, qc * 512:(qc + 1) * 512])
                    nc.sync.dma_start(
                        out[ot * 128:(ot + 1) * 128,
                            qc * 512:(qc + 1) * 512], t_o[:])


# ---------------- host-side sharding / gather ----------------

_CACHED_NC = None


def _get_nc():
    global _CACHED_NC
    if _CACHED_NC is None:
        _CACHED_NC = build_module(reps=1)
    return _CACHED_NC


def _make_in_maps(x, gn_w, gn_b, qkv_w, qkv_b, proj_w, proj_b):
    b, c, h, w = x.shape
    n = h * w
    assert (b, c, n) == (4, C, N)
    xr = np.ascontiguousarray(x.reshape(b, c, n)).astype(np.float32)

    # fp8 weights, prescaled x16.  No 1/sqrt(c) folding: that lives in the
    # exp activation's scale.
    w8_h = np.ascontiguousarray(
        np.concatenate([qkv_w[0:c].T, qkv_w[c:2 * c].T, qkv_w[2 * c:3 * c].T],
                       axis=1) * WSCALE).astype(NP8)
    wp_h = np.ascontiguousarray(proj_w.T * WSCALE).astype(NP8)

    bq_h = np.asarray(qkv_b[0:c], np.float32).reshape(CT, 128)
    bk_h = np.asarray(qkv_b[c:2 * c], np.float32).reshape(CT, 128)
    # v-bias folded through the projection:  proj(attn + bv) =
    # proj(attn) + proj_w @ bv, so it lands in the proj bias.
    bp_eff = (np.asarray(proj_b, np.float64)
              + np.asarray(proj_w, np.float64) @ np.asarray(
                  qkv_b[2 * c:3 * c], np.float64)).astype(np.float32)
    bp_h = bp_eff.reshape(CT, 128)
    gnw_h = np.asarray(gn_w, np.float32).reshape(CT, 128)
    gnb_h = np.asarray(gn_b, np.float32).reshape(CT, 128)
    pidx = np.arange(128)
    indr_h = (pidx[:, None] // GSIZE == np.arange(8)[None, :]).astype(
        np.float32) / GSIZE
    indb_h = (np.arange(8)[:, None] == pidx[None, :] // GSIZE).astype(
        np.float32)
    cvec_h = np.zeros((128, 28), np.float32)
    for ct in range(CT):
        for q, vec in enumerate((bq_h, bk_h, bp_h, gnw_h, gnb_h)):
            cvec_h[:, ct * 5 + q] = vec[ct]
    cvec_h[:, 20:28] = indr_h

    shared = dict(w8=w8_h, wpd=wp_h, cvec=cvec_h, indb=indb_h)
    in_maps = []
    for core in range(N_CORES):
        bi, half = core // 2, core % 2
        xb = xr[bi]
        if half:
            xb = np.ascontiguousarray(
                np.concatenate([xb[:, NQ:], xb[:, :NQ]], axis=1))
        in_maps.append({"xin": xb, **shared})
    return in_maps


def kernel(x, gn_w, gn_b, qkv_w, qkv_b, proj_w, proj_b):
    nc = _get_nc()
    in_maps = _make_in_maps(x, gn_w, gn_b, qkv_w, qkv_b, proj_w, proj_b)
    res = run_bass_kernel_spmd(nc, in_maps, list(range(N_CORES)))
    b, c, h, w = x.shape
    out_full = np.empty((b, C, N), dtype=np.float32)
    for core in range(N_CORES):
        bi, half = core // 2, core % 2
        out_full[bi, :, half * NQ:(half + 1) * NQ] = res.results[core]["out"]
    return out_full.reshape(b, c, h, w)
